# revision 35
# baseline (speedup 1.0000x reference)
"""Trainium2 Bass kernel for zero-phase Butterworth band-stop filter (filtfilt).

Single fused pass: both filtfilt IIR sweeps collapse into one banded
block-Toeplitz convolution with the symmetric autocorrelation kernel
g = h (*) h_rev of the filter impulse response h:

    y[m] = sum_{j=-J..J} F_j @ u[m+j]    (F_j[i,p] = g[i - p - 128 j])

plus two small boundary terms (all matrices host-built in float64):
  * left:  zi transient of pass 1, rank-1 per lane in x0 = ext[Z0]
           (outer-product matmuls with contraction dim 1)
  * right: pass-2 right-edge correction D @ s, where s is the 16-dim
           state (last-8 y1, last-8 u); computed in full fp32 because D
           has ~70x non-normal cancellation. y1's last 8 samples come
           from 3 small fp32 matmuls against unrounded input tails.

All full-width matmuls run in float32r (host-prerounded inputs), which
streams at 1 column/cycle on the PE instead of fp32's 4. Data layout is
block-major ([128 rows = in-block position] x [col = block*4 + lane])
with 8 zero-pad columns on each side, so the shifted operands of F_j are
plain column-offset views of one SBUF tile - no stack DMAs at all.
Output is shipped as bf16 (upcast on host): rounding adds ~2e-3 relmax,
10% of the tolerance, and halves the output DMA bytes.

Sharding: 32 lanes (batch*channel), 4 per NeuronCore across 8 cores.
"""
import os

import numpy as np

import concourse.bacc as bacc
import concourse.mybir as mybir
import concourse.tile as tile
from concourse.bass_utils import run_bass_kernel_spmd

# ---------------- problem geometry (hardcoded for this problem) ----------------
BSH, CSH, T = 4, 8, 131072
LANES = BSH * CSH               # 32
N_CORES = 8
LPC = LANES // N_CORES          # 4 lanes per core
PADLEN = 27
BLK = 128
Z0 = 74                          # front zero padding so ext ends on block edge
L = Z0 + T + 2 * PADLEN          # 131200 samples per lane
NB = L // BLK                    # 1025 blocks per lane
CR = LPC * NB                    # 4100 real columns per core
PF = 8                           # front zero-pad cols (2 blocks)
PB = 8                           # back zero-pad cols
UCOLS = PF + CR + PB             # 4116
NS = 10                          # matmul strips
SW = CR // NS                    # 410 cols per strip (psum bank = 512 f32 max)
JUSE = 1                         # F_j for j in [-JUSE, JUSE]
NF = 2 * JUSE + 1
JORDER = [0, -1, 1, -2, 2][:NF]  # F_0 first so strip matmuls can start early
NWU = 6                          # PE warm-up matmuls (p-state ramp priming)
LH = 640                         # impulse-response length kept
WLB = 2                          # left-zi blocks corrected
DBLK = 3                         # right-edge blocks corrected
NO = 8                           # filter order
OUT_BF16 = True

F32 = mybir.dt.float32
F32R = mybir.dt.float32r
BF16 = mybir.dt.bfloat16
ODT = BF16 if OUT_BF16 else F32

# blob column layout (f32r dram [128, BLOBC])
# SM region: U3 [128,12] | HT [128,24] | s [16,4] (f32, bitcast)
WF_OFF = 0                       # F lhsT   [128, NF*128] f32r
SM_OFF = WF_OFF + NF * BLK
SM_COLS = 36 + 4
WX_OFF = SM_OFF + SM_COLS        # wl lhsT + x0, rows 0:1 [1, 260] f32r
WX_COLS = WLB * BLK + 4
D_OFF = WX_OFF + WX_COLS         # D lhsT rows 0:16 [16, DBLK*128] f32
DS_COLS = DBLK * BLK
UOFF = D_OFF + DS_COLS           # U region [128, UCOLS] f32r (pads zeroed)
BLOBC = UOFF + UCOLS

_matrix_cache: dict = {}
_nc_cache: dict = {}
last_exec_time_ns = None


# ---------------- host-side matrix construction (float64) ----------------
def _round_f32r(v):
    """fp32r pre-rounding: bf16 hi + bf16 lo split (matches device cast)."""
    def bf16(x):
        u = np.ascontiguousarray(x, dtype=np.float32).view(np.uint32)
        return (((u + 0x7FFF + ((u >> 16) & 1)) & 0xFFFF0000)
                .astype(np.uint32)).view(np.float32)
    v32 = np.asarray(v, dtype=np.float32)
    hi = bf16(v32)
    lo = bf16((v32.astype(np.float64) - hi.astype(np.float64)).astype(np.float32))
    return (hi.astype(np.float64) + lo.astype(np.float64)).astype(np.float32)


def _build_matrices(b64, a64):
    key = (b64.tobytes(), a64.tobytes())
    if key in _matrix_cache:
        return _matrix_cache[key]
    bh = b64 / a64[0]
    ah = a64 / a64[0]

    def lfilter1(x):
        y = np.empty_like(x)
        z = np.zeros(NO)
        for t in range(x.shape[0]):
            xt = x[t]
            yt = bh[0] * xt + z[0]
            z[:-1] = z[1:]
            z[-1] = 0.0
            z += bh[1:] * xt - ah[1:] * yt
            y[t] = yt
        return y

    def ar_resp(drive):
        y = np.zeros(drive.shape[0])
        for t in range(y.shape[0]):
            v = drive[t]
            for k in range(1, NO + 1):
                if t - k >= 0:
                    v -= ah[k] * y[t - k]
            y[t] = v
        return y

    imp = np.zeros(LH)
    imp[0] = 1.0
    h = lfilter1(imp)
    g = np.correlate(h, h, mode="full")
    g0 = LH - 1

    ii = np.arange(BLK)[:, None]
    pp = np.arange(BLK)[None, :]
    Fts = []
    for j in JORDER:
        d = ii - pp - BLK * j
        Fj = np.zeros((BLK, BLK))
        mask = np.abs(d) <= (LH - 1)
        Fj[mask] = g[d[mask] + g0]
        Fts.append(Fj.T.copy())

    A = np.zeros((NO, NO))
    A[0] = -ah[1:]
    A[np.arange(1, NO), np.arange(0, NO - 1)] = 1.0
    zi = np.linalg.solve(np.eye(NO) - A.T, bh[1:] - ah[1:] * bh[0])

    # left correction: zi transient of pass 1 through anticausal pass 2
    LT = WLB * BLK
    drive = np.zeros(LT + LH)
    drive[Z0:Z0 + NO] = zi
    t1 = ar_resp(drive)
    wl = np.zeros(LT)
    for t in range(LT):
        wl[t] = np.dot(h, t1[t:t + LH])

    # right correction D [DBLK*128, 16]: s = (y1[L-8..L-1], u[L-8..L-1])
    NTAIL = DBLK * BLK
    D = np.zeros((NTAIL, 16))
    EXT = LH + 16
    for ib in range(16):
        y1t = np.zeros(NO)
        ut = np.zeros(NO)
        if ib < 8:
            y1t[ib] = 1.0
        else:
            ut[ib - 8] = 1.0
        yy = np.zeros(NO + EXT)
        uu = np.zeros(NO + EXT)
        yy[:NO] = y1t
        uu[:NO] = ut
        for t in range(NO, NO + EXT):
            v = 0.0
            for k in range(1, NO + 1):
                v -= ah[k] * yy[t - k]
            for k in range(0, NO + 1):
                if 0 <= t - k < NO:
                    v += bh[k] * uu[t - k]
            yy[t] = v
        ringout = yy[NO:]
        c = np.zeros(NTAIL)
        for idx in range(NTAIL):
            t_off = NTAIL - idx
            kk = np.arange(EXT)
            hidx = kk + t_off
            valid = hidx < LH
            c[idx] = -np.dot(h[hidx[valid]], ringout[valid])
        if ib == 7:                          # zi2 transient, scaled by y1[L-1]
            tr = ar_resp(np.concatenate([zi, np.zeros(NTAIL - NO)]))
            c += tr[NTAIL - 1 - np.arange(NTAIL)]
        D[:, ib] = c

    # Htail_c [8, 128]: y1last8[i] = sum_c Htail_c[i,:] @ u_{NB-1-c}
    HtailT = np.zeros((BLK, 3 * NO))
    for cblk in range(3):
        for i in range(NO):
            for p in range(BLK):
                k = (cblk + 1) * BLK - 1 - (7 - i) - p
                if 0 <= k < LH:
                    HtailT[p, NO * cblk + i] = h[k]

    out = {
        "WF": _round_f32r(np.concatenate(Fts, axis=1)),      # [128, NF*128]
        "HT": HtailT.astype(np.float32),                     # [128, 24]
        "DT": np.concatenate(
            [D[jb * BLK:(jb + 1) * BLK].T for jb in range(DBLK)],
            axis=1).astype(np.float32),                      # [16, DBLK*128]
        "WL": _round_f32r(wl.reshape(1, WLB * BLK)),         # [1, WLB*128]
    }
    _matrix_cache[key] = out
    return out


# ---------------- device kernel ----------------
def _gen_nc():
    nc = bacc.Bacc(None, target_bir_lowering=False)
    blob = nc.dram_tensor("blob", [128, BLOBC], F32R, kind="ExternalInput")
    yout = nc.dram_tensor("y", [128, CR], ODT, kind="ExternalOutput")

    with tile.TileContext(nc) as tc:
        with (
            tc.tile_pool(name="data", bufs=1) as dp,
            tc.tile_pool(name="psum", bufs=7, space="PSUM") as pp,
            tc.tile_pool(name="psumc", bufs=1, space="PSUM") as pc,
        ):
            WF = dp.tile([128, NF * BLK], F32R, tag="WF")
            SMW = dp.tile([128, SM_COLS], F32, tag="SMW")
            WXt = dp.tile([1, WX_COLS], F32R, tag="WX")
            DS = dp.tile([16, DS_COLS], F32, tag="DS")  # D lhsT
            SEG = SW + 16                                # private strip segment
            U = dp.tile([128, NS * SEG], F32R, tag="U")
            Y2 = dp.tile([128, CR], ODT, tag="Y2")
            U3 = SMW[:, 0:12]
            HT = SMW[:, 12:36]
            Svec = SMW[0:16, 36:40]
            WX = WXt[:]

            # weight/small DMAs on sync(SP, HWDGE); U segments split over
            # gpsimd(SWDGE) and scalar(HWDGE), in processing order. Each strip
            # gets a private segment (apron cols re-read from DRAM) so no two
            # input DMAs overlap in SBUF - overlap would chain them serially.
            WU = dp.tile([128, 384], BF16, tag="WU")
            aux = pc.tile([128, 280], F32, tag="aux")
            psv = aux[0:NO, 8:8 + LPC]
            pw = aux[:, 0:NO]
            pd = aux[:, 12:24]
            pwu = aux[:, 24:280]
            nc.vector.memset(WU[:], 0.0)
            for w in range(NWU):
                nc.tensor.matmul(pwu, WU[:, 0:128], WU[:, 128:384],
                                 start=True, stop=True)
            ORDER = [9, 0, 8, 1, 7, 2, 6, 3, 5, 4]
            # U segments for strips fed from both queue families, in
            # processing order: sync(HWDGE) carries the first few odd-position
            # segments interleaved with the weight dmas; gpsimd(SWDGE) the rest
            SYNC_U = {1, 3, 5, 7}
            nc.sync.dma_start(WF[:], blob[:, WF_OFF:WF_OFF + NF * BLK])

            def useg(k):
                return (U[:, SEG * k:SEG * k + SEG],
                        blob[:, UOFF + SW * k:UOFF + SW * k + SEG])

            for i, k in enumerate(ORDER):
                if i not in SYNC_U:
                    nc.gpsimd.dma_start(*useg(k))
            nc.sync.dma_start(useg(ORDER[1])[0], useg(ORDER[1])[1])
            nc.sync.dma_start(SMW[:], blob[:, SM_OFF:SM_OFF + SM_COLS]
                              .bitcast(F32))
            nc.sync.dma_start(WXt[:], blob[0:1, WX_OFF:WX_OFF + WX_COLS])
            nc.sync.dma_start(useg(ORDER[3])[0], useg(ORDER[3])[1])
            nc.sync.dma_start(DS[:], blob[0:16, D_OFF:D_OFF + DS_COLS]
                              .bitcast(F32))
            nc.sync.dma_start(useg(ORDER[5])[0], useg(ORDER[5])[1])
            nc.sync.dma_start(useg(ORDER[7])[0], useg(ORDER[7])[1])

            # out-dma pairs shipped once both member strips are copied
            SHIP = {2: (8, 10, nc.sync), 3: (0, 2, nc.sync),
                    6: (6, 8, nc.sync), 7: (2, 4, nc.sync),
                    9: (4, 6, nc.sync)}
            for i, k in enumerate(ORDER):
                c0, c1 = SW * k, SW * (k + 1)
                u0 = PF + c0
                pm = pp.tile([128, SW], F32, tag="pm")
                ub = SEG * k + 8                     # local col of strip start
                for idx, j in enumerate(JORDER):
                    nc.tensor.matmul(
                        pm[:], WF[:, BLK * idx:BLK * (idx + 1)],
                        U[:, ub + 4 * j:ub + SW + 4 * j],
                        start=(idx == 0), stop=(idx == NF - 1))
                if i == len(ORDER) - 1:
                    h = SW // 2
                    nc.vector.tensor_copy(Y2[:, c0:c0 + h], pm[:, 0:h])
                    nc.scalar.copy(Y2[:, c0 + h:c1], pm[:, h:SW])
                elif i % 2 == 0:
                    nc.vector.tensor_copy(Y2[:, c0:c1], pm[:])
                else:
                    nc.scalar.copy(Y2[:, c0:c1], pm[:])

                if k == NS - 1:
                    # edge paths (all tiny), tucked behind strip 9 on PE
                    for cblk in range(3):
                        nc.tensor.matmul(
                            psv, HT[:, NO * cblk:NO * (cblk + 1)],
                            U3[:, (2 - cblk) * LPC:(3 - cblk) * LPC],
                            start=(cblk == 0), stop=(cblk == 2))
                    nc.vector.tensor_copy(Svec[0:NO, :], psv)
                    for bwl in range(WLB):
                        nc.tensor.matmul(pw[:, LPC * bwl:LPC * (bwl + 1)],
                                         WX[0:1, BLK * bwl:BLK * (bwl + 1)],
                                         WX[0:1, WLB * BLK:WLB * BLK + LPC],
                                         start=True, stop=True)
                    for jb in range(DBLK):
                        nc.tensor.matmul(pd[:, LPC * jb:LPC * (jb + 1)],
                                         DS[:, BLK * jb:BLK * (jb + 1)],
                                         Svec, start=True, stop=True)
                    nc.vector.tensor_add(Y2[:, CR - DBLK * LPC:CR],
                                         Y2[:, CR - DBLK * LPC:CR], pd)
                if k == 0:
                    nc.vector.tensor_add(Y2[:, 0:WLB * LPC],
                                         Y2[:, 0:WLB * LPC], pw)
                if i in SHIP:
                    s0, s1, eng = SHIP[i]
                    eng.dma_start(yout[:, SW * s0:SW * s1],
                                  Y2[:, SW * s0:SW * s1])
    nc.compile()
    return nc


def _get_nc():
    if "nc" not in _nc_cache:
        _nc_cache["nc"] = _gen_nc()
    return _nc_cache["nc"]


def _bf16_to_f32(arr):
    a = np.asarray(arr)
    if a.dtype == np.float32:
        return a
    u = a.view(np.uint16).astype(np.uint32) << 16
    return u.view(np.float32)


# ---------------- host orchestration ----------------
def kernel(x, b=None, a=None):
    global last_exec_time_ns
    x = np.asarray(x)
    in_dtype = x.dtype
    if b is None or a is None:
        raise ValueError("need filter coefficients")
    b64 = np.asarray(b, dtype=np.float64)
    a64 = np.asarray(a, dtype=np.float64)
    W = _build_matrices(b64, a64)

    xl = np.asarray(x, dtype=np.float64).reshape(LANES, T)
    left = 2 * xl[:, :1] - xl[:, PADLEN:0:-1]
    right = 2 * xl[:, -1:] - xl[:, -2:-(PADLEN + 2):-1]
    ext = np.zeros((LANES, L), dtype=np.float32)
    ext[:, Z0:Z0 + PADLEN] = left
    ext[:, Z0 + PADLEN:Z0 + PADLEN + T] = xl
    ext[:, Z0 + PADLEN + T:] = right

    wcols = np.zeros((128, UOFF), dtype=np.float32)
    wcols[:, WF_OFF:WF_OFF + NF * BLK] = W["WF"]
    wcols[:, SM_OFF + 12:SM_OFF + 36] = W["HT"]
    wcols[0:1, WX_OFF:WX_OFF + WLB * BLK] = W["WL"]
    wcols[0:16, D_OFF:D_OFF + DBLK * BLK] = W["DT"]

    in_maps = []
    for core in range(N_CORES):
        lanes = ext[core * LPC:(core + 1) * LPC]             # [LPC, L]
        ublk = lanes.reshape(LPC, NB, BLK).transpose(2, 1, 0).reshape(128, CR)
        blob = np.zeros((128, BLOBC), dtype=np.float32)
        blob[:, :UOFF] = wcols
        blob[:, SM_OFF:SM_OFF + 12] = ublk[:, CR - 12:CR]    # unrounded tails
        blob[8:16, SM_OFF + 36:SM_OFF + 40] = (
            ublk[120:128, CR - LPC:CR])                      # u last-8 per lane
        blob[0:1, WX_OFF + WLB * BLK:WX_OFF + WLB * BLK + LPC] = (
            _round_f32r(lanes[:, Z0]))
        blob[:, UOFF + PF:UOFF + PF + CR] = _round_f32r(ublk)
        in_maps.append({"blob": blob})

    nc = _get_nc()
    trace = bool(int(os.environ.get("BASS_KERNEL_TRACE", "0")))
    res = run_bass_kernel_spmd(nc, in_maps, core_ids=list(range(N_CORES)),
                               trace=trace)
    last_exec_time_ns = res.exec_time_ns

    out = np.empty((LANES, T), dtype=np.float32)
    for core in range(N_CORES):
        ycore = _bf16_to_f32(res.results[core]["y"])         # [128, CR]
        lanes_y = (ycore.reshape(128, NB, LPC).transpose(2, 1, 0)
                   .reshape(LPC, L))
        out[core * LPC:(core + 1) * LPC] = (
            lanes_y[:, Z0 + PADLEN:Z0 + PADLEN + T])
    return out.reshape(BSH, CSH, T).astype(in_dtype)


# revision 36
# speedup vs baseline: 1.0303x; 1.0303x over previous
"""Trainium2 Bass kernel for zero-phase Butterworth band-stop filter (filtfilt).

Single fused pass: both filtfilt IIR sweeps collapse into one banded
block-Toeplitz convolution with the symmetric autocorrelation kernel
g = h (*) h_rev of the filter impulse response h:

    y[m] = sum_{j=-J..J} F_j @ u[m+j]    (F_j[i,p] = g[i - p - 128 j])

plus two small boundary terms (all matrices host-built in float64):
  * left:  zi transient of pass 1, rank-1 per lane in x0 = ext[Z0]
           (outer-product matmuls with contraction dim 1)
  * right: pass-2 right-edge correction D @ s, where s is the 16-dim
           state (last-8 y1, last-8 u); computed in full fp32 because D
           has ~70x non-normal cancellation. y1's last 8 samples come
           from 3 small fp32 matmuls against unrounded input tails.

All full-width matmuls run in float32r (host-prerounded inputs), which
streams at 1 column/cycle on the PE instead of fp32's 4. Data layout is
block-major ([128 rows = in-block position] x [col = block*4 + lane])
with 8 zero-pad columns on each side, so the shifted operands of F_j are
plain column-offset views of one SBUF tile - no stack DMAs at all.
Output is shipped as bf16 (upcast on host): rounding adds ~2e-3 relmax,
10% of the tolerance, and halves the output DMA bytes.

Sharding: 32 lanes (batch*channel), 4 per NeuronCore across 8 cores.
"""
import os

import numpy as np

import concourse.bacc as bacc
import concourse.mybir as mybir
import concourse.tile as tile
from concourse.bass_utils import run_bass_kernel_spmd

# ---------------- problem geometry (hardcoded for this problem) ----------------
BSH, CSH, T = 4, 8, 131072
LANES = BSH * CSH               # 32
N_CORES = 8
LPC = LANES // N_CORES          # 4 lanes per core
PADLEN = 27
BLK = 128
Z0 = 74                          # front zero padding so ext ends on block edge
L = Z0 + T + 2 * PADLEN          # 131200 samples per lane
NB = L // BLK                    # 1025 blocks per lane
CR = LPC * NB                    # 4100 real columns per core
PF = 8                           # front zero-pad cols (2 blocks)
PB = 8                           # back zero-pad cols
UCOLS = PF + CR + PB             # 4116
NS = 10                          # matmul strips
SW = CR // NS                    # 410 cols per strip (psum bank = 512 f32 max)
JUSE = 1                         # F_j for j in [-JUSE, JUSE]
NF = 2 * JUSE + 1
JORDER = [0, -1, 1, -2, 2][:NF]  # F_0 first so strip matmuls can start early
NWU = 6                          # PE warm-up matmuls (p-state ramp priming)
LH = 640                         # impulse-response length kept
WLB = 2                          # left-zi blocks corrected
DBLK = 3                         # right-edge blocks corrected
NO = 8                           # filter order
OUT_BF16 = True

F32 = mybir.dt.float32
F32R = mybir.dt.float32r
BF16 = mybir.dt.bfloat16
ODT = BF16 if OUT_BF16 else F32

# blob column layout (f32r dram [128, BLOBC])
# SM region: U3 [128,12] | HT [128,24] | s [16,4] (f32, bitcast)
WF_OFF = 0                       # F lhsT   [128, NF*128] f32r
SM_OFF = WF_OFF + NF * BLK
SM_COLS = 36 + 4
WX_OFF = SM_OFF + SM_COLS        # wl lhsT + x0, rows 0:1 [1, 260] f32r
WX_COLS = WLB * BLK + 4
D_OFF = WX_OFF + WX_COLS         # D lhsT rows 0:16 [16, DBLK*128] f32
DS_COLS = DBLK * BLK
UOFF = D_OFF + DS_COLS           # U region [128, UCOLS] f32r (pads zeroed)
BLOBC = UOFF + UCOLS

_matrix_cache: dict = {}
_nc_cache: dict = {}
last_exec_time_ns = None


# ---------------- host-side matrix construction (float64) ----------------
def _round_f32r(v):
    """fp32r pre-rounding: bf16 hi + bf16 lo split (matches device cast)."""
    def bf16(x):
        u = np.ascontiguousarray(x, dtype=np.float32).view(np.uint32)
        return (((u + 0x7FFF + ((u >> 16) & 1)) & 0xFFFF0000)
                .astype(np.uint32)).view(np.float32)
    v32 = np.asarray(v, dtype=np.float32)
    hi = bf16(v32)
    lo = bf16((v32.astype(np.float64) - hi.astype(np.float64)).astype(np.float32))
    return (hi.astype(np.float64) + lo.astype(np.float64)).astype(np.float32)


def _build_matrices(b64, a64):
    key = (b64.tobytes(), a64.tobytes())
    if key in _matrix_cache:
        return _matrix_cache[key]
    bh = b64 / a64[0]
    ah = a64 / a64[0]

    def lfilter1(x):
        y = np.empty_like(x)
        z = np.zeros(NO)
        for t in range(x.shape[0]):
            xt = x[t]
            yt = bh[0] * xt + z[0]
            z[:-1] = z[1:]
            z[-1] = 0.0
            z += bh[1:] * xt - ah[1:] * yt
            y[t] = yt
        return y

    def ar_resp(drive):
        y = np.zeros(drive.shape[0])
        for t in range(y.shape[0]):
            v = drive[t]
            for k in range(1, NO + 1):
                if t - k >= 0:
                    v -= ah[k] * y[t - k]
            y[t] = v
        return y

    imp = np.zeros(LH)
    imp[0] = 1.0
    h = lfilter1(imp)
    g = np.correlate(h, h, mode="full")
    g0 = LH - 1

    ii = np.arange(BLK)[:, None]
    pp = np.arange(BLK)[None, :]
    Fts = []
    for j in JORDER:
        d = ii - pp - BLK * j
        Fj = np.zeros((BLK, BLK))
        mask = np.abs(d) <= (LH - 1)
        Fj[mask] = g[d[mask] + g0]
        Fts.append(Fj.T.copy())

    A = np.zeros((NO, NO))
    A[0] = -ah[1:]
    A[np.arange(1, NO), np.arange(0, NO - 1)] = 1.0
    zi = np.linalg.solve(np.eye(NO) - A.T, bh[1:] - ah[1:] * bh[0])

    # left correction: zi transient of pass 1 through anticausal pass 2
    LT = WLB * BLK
    drive = np.zeros(LT + LH)
    drive[Z0:Z0 + NO] = zi
    t1 = ar_resp(drive)
    wl = np.zeros(LT)
    for t in range(LT):
        wl[t] = np.dot(h, t1[t:t + LH])

    # right correction D [DBLK*128, 16]: s = (y1[L-8..L-1], u[L-8..L-1])
    NTAIL = DBLK * BLK
    D = np.zeros((NTAIL, 16))
    EXT = LH + 16
    for ib in range(16):
        y1t = np.zeros(NO)
        ut = np.zeros(NO)
        if ib < 8:
            y1t[ib] = 1.0
        else:
            ut[ib - 8] = 1.0
        yy = np.zeros(NO + EXT)
        uu = np.zeros(NO + EXT)
        yy[:NO] = y1t
        uu[:NO] = ut
        for t in range(NO, NO + EXT):
            v = 0.0
            for k in range(1, NO + 1):
                v -= ah[k] * yy[t - k]
            for k in range(0, NO + 1):
                if 0 <= t - k < NO:
                    v += bh[k] * uu[t - k]
            yy[t] = v
        ringout = yy[NO:]
        c = np.zeros(NTAIL)
        for idx in range(NTAIL):
            t_off = NTAIL - idx
            kk = np.arange(EXT)
            hidx = kk + t_off
            valid = hidx < LH
            c[idx] = -np.dot(h[hidx[valid]], ringout[valid])
        if ib == 7:                          # zi2 transient, scaled by y1[L-1]
            tr = ar_resp(np.concatenate([zi, np.zeros(NTAIL - NO)]))
            c += tr[NTAIL - 1 - np.arange(NTAIL)]
        D[:, ib] = c

    # Htail_c [8, 128]: y1last8[i] = sum_c Htail_c[i,:] @ u_{NB-1-c}
    HtailT = np.zeros((BLK, 3 * NO))
    for cblk in range(3):
        for i in range(NO):
            for p in range(BLK):
                k = (cblk + 1) * BLK - 1 - (7 - i) - p
                if 0 <= k < LH:
                    HtailT[p, NO * cblk + i] = h[k]

    out = {
        "WF": _round_f32r(np.concatenate(Fts, axis=1)),      # [128, NF*128]
        "HT": HtailT.astype(np.float32),                     # [128, 24]
        "DT": np.concatenate(
            [D[jb * BLK:(jb + 1) * BLK].T for jb in range(DBLK)],
            axis=1).astype(np.float32),                      # [16, DBLK*128]
        "WL": _round_f32r(wl.reshape(1, WLB * BLK)),         # [1, WLB*128]
    }
    _matrix_cache[key] = out
    return out


# ---------------- device kernel ----------------
def _gen_nc():
    nc = bacc.Bacc(None, target_bir_lowering=False)
    blob = nc.dram_tensor("blob", [128, BLOBC], F32R, kind="ExternalInput")
    yout = nc.dram_tensor("y", [128, CR], ODT, kind="ExternalOutput")

    with tile.TileContext(nc) as tc:
        with (
            tc.tile_pool(name="data", bufs=1) as dp,
            tc.tile_pool(name="psum", bufs=7, space="PSUM") as pp,
            tc.tile_pool(name="psumc", bufs=1, space="PSUM") as pc,
        ):
            WF = dp.tile([128, NF * BLK], F32R, tag="WF")
            SMW = dp.tile([128, SM_COLS], F32, tag="SMW")
            WXt = dp.tile([1, WX_COLS], F32R, tag="WX")
            DS = dp.tile([16, DS_COLS], F32, tag="DS")  # D lhsT
            SEG = SW + 16                                # private strip segment
            U = dp.tile([128, NS * SEG], F32R, tag="U")
            Y2 = dp.tile([128, CR], ODT, tag="Y2")
            U3 = SMW[:, 0:12]
            HT = SMW[:, 12:36]
            Svec = SMW[0:16, 36:40]
            WX = WXt[:]

            # weight/small DMAs on sync(SP, HWDGE); U segments split over
            # gpsimd(SWDGE) and scalar(HWDGE), in processing order. Each strip
            # gets a private segment (apron cols re-read from DRAM) so no two
            # input DMAs overlap in SBUF - overlap would chain them serially.
            WU = dp.tile([128, 384], BF16, tag="WU")
            aux = pc.tile([128, 280], F32, tag="aux")
            psv = aux[0:NO, 8:8 + LPC]
            pw = aux[:, 0:NO]
            pd = aux[:, 12:24]
            pwu = aux[:, 24:280]
            nc.vector.memset(WU[:], 0.0)
            for w in range(NWU):
                nc.tensor.matmul(pwu, WU[:, 0:128], WU[:, 128:384],
                                 start=True, stop=True)
            ORDER = [9, 0, 8, 1, 7, 2, 6, 3, 5, 4]
            # U segments for strips fed from both queue families, in
            # processing order: sync(HWDGE) carries the first few odd-position
            # segments interleaved with the weight dmas; gpsimd(SWDGE) the rest
            SYNC_U = {1, 3, 5, 7}
            nc.sync.dma_start(WF[:], blob[:, WF_OFF:WF_OFF + NF * BLK])

            def useg(k):
                return (U[:, SEG * k:SEG * k + SEG],
                        blob[:, UOFF + SW * k:UOFF + SW * k + SEG])

            for i, k in enumerate(ORDER):
                if i not in SYNC_U:
                    nc.gpsimd.dma_start(*useg(k))
            nc.sync.dma_start(useg(ORDER[1])[0], useg(ORDER[1])[1])
            nc.sync.dma_start(SMW[:], blob[:, SM_OFF:SM_OFF + SM_COLS]
                              .bitcast(F32))
            nc.sync.dma_start(WXt[:], blob[0:1, WX_OFF:WX_OFF + WX_COLS])
            nc.sync.dma_start(useg(ORDER[3])[0], useg(ORDER[3])[1])
            nc.sync.dma_start(DS[:], blob[0:16, D_OFF:D_OFF + DS_COLS]
                              .bitcast(F32))
            nc.sync.dma_start(useg(ORDER[5])[0], useg(ORDER[5])[1])
            nc.sync.dma_start(useg(ORDER[7])[0], useg(ORDER[7])[1])

            # out-dma pairs shipped once both member strips are copied
            SHIP = {2: (8, 10, nc.sync), 3: (0, 2, nc.sync),
                    6: (6, 8, nc.sync), 7: (2, 4, nc.sync),
                    9: (4, 6, nc.sync)}
            for i, k in enumerate(ORDER):
                c0, c1 = SW * k, SW * (k + 1)
                u0 = PF + c0
                pm = pp.tile([128, SW], F32, tag="pm")
                ub = SEG * k + 8                     # local col of strip start
                for idx, j in enumerate(JORDER):
                    nc.tensor.matmul(
                        pm[:], WF[:, BLK * idx:BLK * (idx + 1)],
                        U[:, ub + 4 * j:ub + SW + 4 * j],
                        start=(idx == 0), stop=(idx == NF - 1))
                if i % 2 == 0:
                    nc.vector.tensor_copy(Y2[:, c0:c1], pm[:])
                else:
                    nc.scalar.copy(Y2[:, c0:c1], pm[:])

                if k == NS - 1:
                    # edge paths (all tiny), tucked behind strip 9 on PE
                    for cblk in range(3):
                        nc.tensor.matmul(
                            psv, HT[:, NO * cblk:NO * (cblk + 1)],
                            U3[:, (2 - cblk) * LPC:(3 - cblk) * LPC],
                            start=(cblk == 0), stop=(cblk == 2))
                    nc.vector.tensor_copy(Svec[0:NO, :], psv)
                    for bwl in range(WLB):
                        nc.tensor.matmul(pw[:, LPC * bwl:LPC * (bwl + 1)],
                                         WX[0:1, BLK * bwl:BLK * (bwl + 1)],
                                         WX[0:1, WLB * BLK:WLB * BLK + LPC],
                                         start=True, stop=True)
                    for jb in range(DBLK):
                        nc.tensor.matmul(pd[:, LPC * jb:LPC * (jb + 1)],
                                         DS[:, BLK * jb:BLK * (jb + 1)],
                                         Svec, start=True, stop=True)
                    nc.vector.tensor_add(Y2[:, CR - DBLK * LPC:CR],
                                         Y2[:, CR - DBLK * LPC:CR], pd)
                if k == 0:
                    nc.vector.tensor_add(Y2[:, 0:WLB * LPC],
                                         Y2[:, 0:WLB * LPC], pw)
                if i in SHIP:
                    s0, s1, eng = SHIP[i]
                    eng.dma_start(yout[:, SW * s0:SW * s1],
                                  Y2[:, SW * s0:SW * s1])
    nc.compile()
    return nc


def _get_nc():
    if "nc" not in _nc_cache:
        _nc_cache["nc"] = _gen_nc()
    return _nc_cache["nc"]


def _bf16_to_f32(arr):
    a = np.asarray(arr)
    if a.dtype == np.float32:
        return a
    u = a.view(np.uint16).astype(np.uint32) << 16
    return u.view(np.float32)


# ---------------- host orchestration ----------------
def kernel(x, b=None, a=None):
    global last_exec_time_ns
    x = np.asarray(x)
    in_dtype = x.dtype
    if b is None or a is None:
        raise ValueError("need filter coefficients")
    b64 = np.asarray(b, dtype=np.float64)
    a64 = np.asarray(a, dtype=np.float64)
    W = _build_matrices(b64, a64)

    xl = np.asarray(x, dtype=np.float64).reshape(LANES, T)
    left = 2 * xl[:, :1] - xl[:, PADLEN:0:-1]
    right = 2 * xl[:, -1:] - xl[:, -2:-(PADLEN + 2):-1]
    ext = np.zeros((LANES, L), dtype=np.float32)
    ext[:, Z0:Z0 + PADLEN] = left
    ext[:, Z0 + PADLEN:Z0 + PADLEN + T] = xl
    ext[:, Z0 + PADLEN + T:] = right

    wcols = np.zeros((128, UOFF), dtype=np.float32)
    wcols[:, WF_OFF:WF_OFF + NF * BLK] = W["WF"]
    wcols[:, SM_OFF + 12:SM_OFF + 36] = W["HT"]
    wcols[0:1, WX_OFF:WX_OFF + WLB * BLK] = W["WL"]
    wcols[0:16, D_OFF:D_OFF + DBLK * BLK] = W["DT"]

    in_maps = []
    for core in range(N_CORES):
        lanes = ext[core * LPC:(core + 1) * LPC]             # [LPC, L]
        ublk = lanes.reshape(LPC, NB, BLK).transpose(2, 1, 0).reshape(128, CR)
        blob = np.zeros((128, BLOBC), dtype=np.float32)
        blob[:, :UOFF] = wcols
        blob[:, SM_OFF:SM_OFF + 12] = ublk[:, CR - 12:CR]    # unrounded tails
        blob[8:16, SM_OFF + 36:SM_OFF + 40] = (
            ublk[120:128, CR - LPC:CR])                      # u last-8 per lane
        blob[0:1, WX_OFF + WLB * BLK:WX_OFF + WLB * BLK + LPC] = (
            _round_f32r(lanes[:, Z0]))
        blob[:, UOFF + PF:UOFF + PF + CR] = _round_f32r(ublk)
        in_maps.append({"blob": blob})

    nc = _get_nc()
    trace = bool(int(os.environ.get("BASS_KERNEL_TRACE", "0")))
    res = run_bass_kernel_spmd(nc, in_maps, core_ids=list(range(N_CORES)),
                               trace=trace)
    last_exec_time_ns = res.exec_time_ns

    out = np.empty((LANES, T), dtype=np.float32)
    for core in range(N_CORES):
        ycore = _bf16_to_f32(res.results[core]["y"])         # [128, CR]
        lanes_y = (ycore.reshape(128, NB, LPC).transpose(2, 1, 0)
                   .reshape(LPC, L))
        out[core * LPC:(core + 1) * LPC] = (
            lanes_y[:, Z0 + PADLEN:Z0 + PADLEN + T])
    return out.reshape(BSH, CSH, T).astype(in_dtype)


# revision 38
# speedup vs baseline: 1.0339x; 1.0034x over previous
"""Trainium2 Bass kernel for zero-phase Butterworth band-stop filter (filtfilt).

Single fused pass: both filtfilt IIR sweeps collapse into one banded
block-Toeplitz convolution with the symmetric autocorrelation kernel
g = h (*) h_rev of the filter impulse response h:

    y[m] = sum_{j=-J..J} F_j @ u[m+j]    (F_j[i,p] = g[i - p - 128 j])

plus two small boundary terms (all matrices host-built in float64):
  * left:  zi transient of pass 1, rank-1 per lane in x0 = ext[Z0]
           (outer-product matmuls with contraction dim 1)
  * right: pass-2 right-edge correction D @ s, where s is the 16-dim
           state (last-8 y1, last-8 u); computed in full fp32 because D
           has ~70x non-normal cancellation. y1's last 8 samples come
           from 3 small fp32 matmuls against unrounded input tails.

All full-width matmuls run in float32r (host-prerounded inputs), which
streams at 1 column/cycle on the PE instead of fp32's 4. Data layout is
block-major ([128 rows = in-block position] x [col = block*4 + lane])
with 8 zero-pad columns on each side, so the shifted operands of F_j are
plain column-offset views of one SBUF tile - no stack DMAs at all.
Output is shipped as bf16 (upcast on host): rounding adds ~2e-3 relmax,
10% of the tolerance, and halves the output DMA bytes.

Sharding: 32 lanes (batch*channel), 4 per NeuronCore across 8 cores.
"""
import os

import numpy as np

import concourse.bacc as bacc
import concourse.mybir as mybir
import concourse.tile as tile
from concourse.bass_utils import run_bass_kernel_spmd

# ---------------- problem geometry (hardcoded for this problem) ----------------
BSH, CSH, T = 4, 8, 131072
LANES = BSH * CSH               # 32
N_CORES = 8
LPC = LANES // N_CORES          # 4 lanes per core
PADLEN = 27
BLK = 128
Z0 = 74                          # front zero padding so ext ends on block edge
L = Z0 + T + 2 * PADLEN          # 131200 samples per lane
NB = L // BLK                    # 1025 blocks per lane
CR = LPC * NB                    # 4100 real columns per core
PF = 8                           # front zero-pad cols (2 blocks)
PB = 8                           # back zero-pad cols
UCOLS = PF + CR + PB             # 4116
# column-ordered strip widths (psum bank max 512 f32); s4 is narrow (>=256
# keeps f32r at 1 cyc/col) and is processed last, shortening the tail chain
WIDTHS = [428, 428, 428, 428, 260, 428, 428, 428, 428, 416]
NS = len(WIDTHS)
CUM = [0]
for _w in WIDTHS:
    CUM.append(CUM[-1] + _w)
assert CUM[-1] == CR
JUSE = 1                         # F_j for j in [-JUSE, JUSE]
NF = 2 * JUSE + 1
JORDER = [0, -1, 1, -2, 2][:NF]  # F_0 first so strip matmuls can start early
NWU = 6                          # PE warm-up matmuls (p-state ramp priming)
LH = 640                         # impulse-response length kept
WLB = 2                          # left-zi blocks corrected
DBLK = 3                         # right-edge blocks corrected
NO = 8                           # filter order
OUT_BF16 = True

F32 = mybir.dt.float32
F32R = mybir.dt.float32r
BF16 = mybir.dt.bfloat16
ODT = BF16 if OUT_BF16 else F32

# blob column layout (f32r dram [128, BLOBC])
# SM region: U3 [128,12] | HT [128,24] | s [16,4] (f32, bitcast)
WF_OFF = 0                       # F lhsT   [128, NF*128] f32r
SM_OFF = WF_OFF + NF * BLK
SM_COLS = 36 + 4
WX_OFF = SM_OFF + SM_COLS        # wl lhsT + x0, rows 0:1 [1, 260] f32r
WX_COLS = WLB * BLK + 4
D_OFF = WX_OFF + WX_COLS         # D lhsT rows 0:16 [16, DBLK*128] f32
DS_COLS = DBLK * BLK
UOFF = D_OFF + DS_COLS           # U region [128, UCOLS] f32r (pads zeroed)
BLOBC = UOFF + UCOLS

_matrix_cache: dict = {}
_nc_cache: dict = {}
last_exec_time_ns = None


# ---------------- host-side matrix construction (float64) ----------------
def _round_f32r(v):
    """fp32r pre-rounding: bf16 hi + bf16 lo split (matches device cast)."""
    def bf16(x):
        u = np.ascontiguousarray(x, dtype=np.float32).view(np.uint32)
        return (((u + 0x7FFF + ((u >> 16) & 1)) & 0xFFFF0000)
                .astype(np.uint32)).view(np.float32)
    v32 = np.asarray(v, dtype=np.float32)
    hi = bf16(v32)
    lo = bf16((v32.astype(np.float64) - hi.astype(np.float64)).astype(np.float32))
    return (hi.astype(np.float64) + lo.astype(np.float64)).astype(np.float32)


def _build_matrices(b64, a64):
    key = (b64.tobytes(), a64.tobytes())
    if key in _matrix_cache:
        return _matrix_cache[key]
    bh = b64 / a64[0]
    ah = a64 / a64[0]

    def lfilter1(x):
        y = np.empty_like(x)
        z = np.zeros(NO)
        for t in range(x.shape[0]):
            xt = x[t]
            yt = bh[0] * xt + z[0]
            z[:-1] = z[1:]
            z[-1] = 0.0
            z += bh[1:] * xt - ah[1:] * yt
            y[t] = yt
        return y

    def ar_resp(drive):
        y = np.zeros(drive.shape[0])
        for t in range(y.shape[0]):
            v = drive[t]
            for k in range(1, NO + 1):
                if t - k >= 0:
                    v -= ah[k] * y[t - k]
            y[t] = v
        return y

    imp = np.zeros(LH)
    imp[0] = 1.0
    h = lfilter1(imp)
    g = np.correlate(h, h, mode="full")
    g0 = LH - 1

    ii = np.arange(BLK)[:, None]
    pp = np.arange(BLK)[None, :]
    Fts = []
    for j in JORDER:
        d = ii - pp - BLK * j
        Fj = np.zeros((BLK, BLK))
        mask = np.abs(d) <= (LH - 1)
        Fj[mask] = g[d[mask] + g0]
        Fts.append(Fj.T.copy())

    A = np.zeros((NO, NO))
    A[0] = -ah[1:]
    A[np.arange(1, NO), np.arange(0, NO - 1)] = 1.0
    zi = np.linalg.solve(np.eye(NO) - A.T, bh[1:] - ah[1:] * bh[0])

    # left correction: zi transient of pass 1 through anticausal pass 2
    LT = WLB * BLK
    drive = np.zeros(LT + LH)
    drive[Z0:Z0 + NO] = zi
    t1 = ar_resp(drive)
    wl = np.zeros(LT)
    for t in range(LT):
        wl[t] = np.dot(h, t1[t:t + LH])

    # right correction D [DBLK*128, 16]: s = (y1[L-8..L-1], u[L-8..L-1])
    NTAIL = DBLK * BLK
    D = np.zeros((NTAIL, 16))
    EXT = LH + 16
    for ib in range(16):
        y1t = np.zeros(NO)
        ut = np.zeros(NO)
        if ib < 8:
            y1t[ib] = 1.0
        else:
            ut[ib - 8] = 1.0
        yy = np.zeros(NO + EXT)
        uu = np.zeros(NO + EXT)
        yy[:NO] = y1t
        uu[:NO] = ut
        for t in range(NO, NO + EXT):
            v = 0.0
            for k in range(1, NO + 1):
                v -= ah[k] * yy[t - k]
            for k in range(0, NO + 1):
                if 0 <= t - k < NO:
                    v += bh[k] * uu[t - k]
            yy[t] = v
        ringout = yy[NO:]
        c = np.zeros(NTAIL)
        for idx in range(NTAIL):
            t_off = NTAIL - idx
            kk = np.arange(EXT)
            hidx = kk + t_off
            valid = hidx < LH
            c[idx] = -np.dot(h[hidx[valid]], ringout[valid])
        if ib == 7:                          # zi2 transient, scaled by y1[L-1]
            tr = ar_resp(np.concatenate([zi, np.zeros(NTAIL - NO)]))
            c += tr[NTAIL - 1 - np.arange(NTAIL)]
        D[:, ib] = c

    # Htail_c [8, 128]: y1last8[i] = sum_c Htail_c[i,:] @ u_{NB-1-c}
    HtailT = np.zeros((BLK, 3 * NO))
    for cblk in range(3):
        for i in range(NO):
            for p in range(BLK):
                k = (cblk + 1) * BLK - 1 - (7 - i) - p
                if 0 <= k < LH:
                    HtailT[p, NO * cblk + i] = h[k]

    out = {
        "WF": _round_f32r(np.concatenate(Fts, axis=1)),      # [128, NF*128]
        "HT": HtailT.astype(np.float32),                     # [128, 24]
        "DT": np.concatenate(
            [D[jb * BLK:(jb + 1) * BLK].T for jb in range(DBLK)],
            axis=1).astype(np.float32),                      # [16, DBLK*128]
        "WL": _round_f32r(wl.reshape(1, WLB * BLK)),         # [1, WLB*128]
    }
    _matrix_cache[key] = out
    return out


# ---------------- device kernel ----------------
def _gen_nc():
    nc = bacc.Bacc(None, target_bir_lowering=False)
    blob = nc.dram_tensor("blob", [128, BLOBC], F32R, kind="ExternalInput")
    yout = nc.dram_tensor("y", [128, CR], ODT, kind="ExternalOutput")

    with tile.TileContext(nc) as tc:
        with (
            tc.tile_pool(name="data", bufs=1) as dp,
            tc.tile_pool(name="psum", bufs=7, space="PSUM") as pp,
            tc.tile_pool(name="psumc", bufs=1, space="PSUM") as pc,
        ):
            WF = dp.tile([128, NF * BLK], F32R, tag="WF")
            SMW = dp.tile([128, SM_COLS], F32, tag="SMW")
            WXt = dp.tile([1, WX_COLS], F32R, tag="WX")
            DS = dp.tile([16, DS_COLS], F32, tag="DS")  # D lhsT
            U = dp.tile([128, CR + 16 * NS], F32R, tag="U")
            Y2 = dp.tile([128, CR], ODT, tag="Y2")
            U3 = SMW[:, 0:12]
            HT = SMW[:, 12:36]
            Svec = SMW[0:16, 36:40]
            WX = WXt[:]

            # weight/small DMAs on sync(SP, HWDGE); U segments split over
            # gpsimd(SWDGE) and scalar(HWDGE), in processing order. Each strip
            # gets a private segment (apron cols re-read from DRAM) so no two
            # input DMAs overlap in SBUF - overlap would chain them serially.
            WU = dp.tile([128, 384], BF16, tag="WU")
            aux = pc.tile([128, 280], F32, tag="aux")
            psv = aux[0:NO, 8:8 + LPC]
            pw = aux[:, 0:NO]
            pd = aux[:, 12:24]
            pwu = aux[:, 24:280]
            nc.vector.memset(WU[:], 0.0)
            for w in range(NWU):
                nc.tensor.matmul(pwu, WU[:, 0:128], WU[:, 128:384],
                                 start=True, stop=True)
            ORDER = [9, 0, 8, 1, 7, 2, 6, 3, 5, 4]
            SEGB = [CUM[k] + 16 * k for k in range(NS)]
            # U segments for strips fed from both queue families, in
            # processing order: sync(HWDGE) carries the first few odd-position
            # segments interleaved with the weight dmas; gpsimd(SWDGE) the rest
            SYNC_U = {1, 3, 5, 7}
            nc.sync.dma_start(WF[:], blob[:, WF_OFF:WF_OFF + NF * BLK])

            def useg(k):
                w16 = WIDTHS[k] + 16
                return (U[:, SEGB[k]:SEGB[k] + w16],
                        blob[:, UOFF + CUM[k]:UOFF + CUM[k] + w16])

            for i, k in enumerate(ORDER):
                if i not in SYNC_U:
                    nc.gpsimd.dma_start(*useg(k))
            nc.sync.dma_start(useg(ORDER[1])[0], useg(ORDER[1])[1])
            nc.sync.dma_start(SMW[:], blob[:, SM_OFF:SM_OFF + SM_COLS]
                              .bitcast(F32))
            nc.sync.dma_start(WXt[:], blob[0:1, WX_OFF:WX_OFF + WX_COLS])
            nc.sync.dma_start(useg(ORDER[3])[0], useg(ORDER[3])[1])
            nc.sync.dma_start(DS[:], blob[0:16, D_OFF:D_OFF + DS_COLS]
                              .bitcast(F32))
            nc.sync.dma_start(useg(ORDER[5])[0], useg(ORDER[5])[1])
            nc.sync.dma_start(useg(ORDER[7])[0], useg(ORDER[7])[1])

            # out-dma pairs shipped once both member strips are copied
            SHIP = {2: (CUM[8], CUM[10]), 3: (CUM[0], CUM[2]),
                    6: (CUM[6], CUM[8]), 7: (CUM[2], CUM[4]),
                    9: (CUM[4], CUM[6])}
            for i, k in enumerate(ORDER):
                c0, c1 = CUM[k], CUM[k + 1]
                w = WIDTHS[k]
                pm = pp.tile([128, 512], F32, tag="pm")
                ub = SEGB[k] + 8                     # local col of strip start
                for idx, j in enumerate(JORDER):
                    nc.tensor.matmul(
                        pm[:, 0:w], WF[:, BLK * idx:BLK * (idx + 1)],
                        U[:, ub + 4 * j:ub + w + 4 * j],
                        start=(idx == 0), stop=(idx == NF - 1))
                if i % 2 == 0:
                    nc.vector.tensor_copy(Y2[:, c0:c1], pm[:, 0:w])
                else:
                    nc.scalar.copy(Y2[:, c0:c1], pm[:, 0:w])

                if k == NS - 1:
                    # edge paths (all tiny), tucked behind strip 9 on PE
                    for cblk in range(3):
                        nc.tensor.matmul(
                            psv, HT[:, NO * cblk:NO * (cblk + 1)],
                            U3[:, (2 - cblk) * LPC:(3 - cblk) * LPC],
                            start=(cblk == 0), stop=(cblk == 2))
                    nc.vector.tensor_copy(Svec[0:NO, :], psv)
                    for bwl in range(WLB):
                        nc.tensor.matmul(pw[:, LPC * bwl:LPC * (bwl + 1)],
                                         WX[0:1, BLK * bwl:BLK * (bwl + 1)],
                                         WX[0:1, WLB * BLK:WLB * BLK + LPC],
                                         start=True, stop=True)
                    for jb in range(DBLK):
                        nc.tensor.matmul(pd[:, LPC * jb:LPC * (jb + 1)],
                                         DS[:, BLK * jb:BLK * (jb + 1)],
                                         Svec, start=True, stop=True)
                    nc.vector.tensor_add(Y2[:, CR - DBLK * LPC:CR],
                                         Y2[:, CR - DBLK * LPC:CR], pd)
                if k == 0:
                    nc.vector.tensor_add(Y2[:, 0:WLB * LPC],
                                         Y2[:, 0:WLB * LPC], pw)
                if i in SHIP:
                    s0, s1 = SHIP[i]
                    nc.sync.dma_start(yout[:, s0:s1], Y2[:, s0:s1])
    nc.compile()
    return nc


def _get_nc():
    if "nc" not in _nc_cache:
        _nc_cache["nc"] = _gen_nc()
    return _nc_cache["nc"]


def _bf16_to_f32(arr):
    a = np.asarray(arr)
    if a.dtype == np.float32:
        return a
    u = a.view(np.uint16).astype(np.uint32) << 16
    return u.view(np.float32)


# ---------------- host orchestration ----------------
def kernel(x, b=None, a=None):
    global last_exec_time_ns
    x = np.asarray(x)
    in_dtype = x.dtype
    if b is None or a is None:
        raise ValueError("need filter coefficients")
    b64 = np.asarray(b, dtype=np.float64)
    a64 = np.asarray(a, dtype=np.float64)
    W = _build_matrices(b64, a64)

    xl = np.asarray(x, dtype=np.float64).reshape(LANES, T)
    left = 2 * xl[:, :1] - xl[:, PADLEN:0:-1]
    right = 2 * xl[:, -1:] - xl[:, -2:-(PADLEN + 2):-1]
    ext = np.zeros((LANES, L), dtype=np.float32)
    ext[:, Z0:Z0 + PADLEN] = left
    ext[:, Z0 + PADLEN:Z0 + PADLEN + T] = xl
    ext[:, Z0 + PADLEN + T:] = right

    wcols = np.zeros((128, UOFF), dtype=np.float32)
    wcols[:, WF_OFF:WF_OFF + NF * BLK] = W["WF"]
    wcols[:, SM_OFF + 12:SM_OFF + 36] = W["HT"]
    wcols[0:1, WX_OFF:WX_OFF + WLB * BLK] = W["WL"]
    wcols[0:16, D_OFF:D_OFF + DBLK * BLK] = W["DT"]

    in_maps = []
    for core in range(N_CORES):
        lanes = ext[core * LPC:(core + 1) * LPC]             # [LPC, L]
        ublk = lanes.reshape(LPC, NB, BLK).transpose(2, 1, 0).reshape(128, CR)
        blob = np.zeros((128, BLOBC), dtype=np.float32)
        blob[:, :UOFF] = wcols
        blob[:, SM_OFF:SM_OFF + 12] = ublk[:, CR - 12:CR]    # unrounded tails
        blob[8:16, SM_OFF + 36:SM_OFF + 40] = (
            ublk[120:128, CR - LPC:CR])                      # u last-8 per lane
        blob[0:1, WX_OFF + WLB * BLK:WX_OFF + WLB * BLK + LPC] = (
            _round_f32r(lanes[:, Z0]))
        blob[:, UOFF + PF:UOFF + PF + CR] = _round_f32r(ublk)
        in_maps.append({"blob": blob})

    nc = _get_nc()
    trace = bool(int(os.environ.get("BASS_KERNEL_TRACE", "0")))
    res = run_bass_kernel_spmd(nc, in_maps, core_ids=list(range(N_CORES)),
                               trace=trace)
    last_exec_time_ns = res.exec_time_ns

    out = np.empty((LANES, T), dtype=np.float32)
    for core in range(N_CORES):
        ycore = _bf16_to_f32(res.results[core]["y"])         # [128, CR]
        lanes_y = (ycore.reshape(128, NB, LPC).transpose(2, 1, 0)
                   .reshape(LPC, L))
        out[core * LPC:(core + 1) * LPC] = (
            lanes_y[:, Z0 + PADLEN:Z0 + PADLEN + T])
    return out.reshape(BSH, CSH, T).astype(in_dtype)
